# revision 1
# baseline (speedup 1.0000x reference)
"""Trainium2 Bass kernel for nn_DualEncoderModel — v2 (staircase + hybrid gather).

Structure (8 cores, 8 batches/core, pairs sorted by u on host):
  phase 1 (all batches): encoder matmuls -> EMB [64ch, 256ag]; per-agent table
    TBL [128ag-rows, 256] = [emb_f|G_f | emb_u|G_u] (G folds the product trick
    p=ef*eu via squares); f-half written to DRAM as the 256B-row gather table;
    u-half pre-DIFFERENCED along agents (W~[p] = W[p]-W[p+1]) so the u-side
    "gather" is a matmul with the suffix indicator A[p,j] = (j < end_p).
  phase 2 (per batch, 4 chunks of 1024 pairs):
    A     = is_lt(IOTA, end)                   (DVE tensor_scalar, 4x)
    P1    = W~u^T A            -> [eu; gu]     (PE, telescoping sum)
    D     = GF[0:64] - P1[0:64]                (DVE, ef - eu)
    P1   += I128 * GF          -> [s; g]       (PE identity-add)
    NL[0:64]  = Square(P1[0:64])               (ACT fused evict: s^2)
    NL[64:128]= abs_max(D, 0)                  (DVE 4x: |d|)
    P1[64:]  += [wd2; C]^T NL  -> h            (PE)
    WH    = Relu(P1[64:] + b1)                 (ACT)
    P_L[c]= w2^T WH                            (PE)
  GF [128, 4096] = [ef; gf] comes from a dma_gather (transpose) of the f-table
  with the u-sorted f-index stream. Host sorts pairs by u per batch, ships the
  per-agent run ends, and unpermutes the logits.
"""

import os
import sys

import numpy as np

for _p in ("/opt/trn_rl_repo", "/root/.axon_site/_ro/trn_rl_repo"):
    if _p not in sys.path and os.path.isdir(_p):
        sys.path.insert(0, _p)

import concourse.bass as bass
import concourse.bacc as bacc
import concourse.tile as tile
from concourse import mybir
from concourse.bass_utils import run_bass_kernel_spmd

B, L, A, F, E, P = 64, 50, 256, 8, 64, 4096
NF = A // 2
NCORES = 8
BPC = B // NCORES

dt = mybir.dt
F16 = dt.float16
F32 = dt.float32
AF = mybir.ActivationFunctionType
ALU = mybir.AluOpType

CH = 1024           # pairs per chunk
NCH = P // CH       # 4 chunks per batch
GPC = 4             # gather pieces per batch
GW = P // GPC       # idxs per gather piece


def build_program(bpc=BPC):
    nc = bacc.Bacc("TRN2", target_bir_lowering=False, debug=False)

    traj = nc.dram_tensor("traj", [bpc, L, A * F], F16, kind="ExternalInput")
    idxf = nc.dram_tensor("idxf", [128, (P // 16) * bpc], dt.int16, kind="ExternalInput")
    wenc = nc.dram_tensor("wenc", [L, 8 * 128], F16, kind="ExternalInput")
    wg = nc.dram_tensor("wg", [64, 256], F16, kind="ExternalInput")
    ident = nc.dram_tensor("ident", [64, 64], F16, kind="ExternalInput")
    mdiff = nc.dram_tensor("mdiff", [128, 128], F16, kind="ExternalInput")
    i128 = nc.dram_tensor("i128", [128, 128], F16, kind="ExternalInput")
    wnl = nc.dram_tensor("wnl", [128, 64], F16, kind="ExternalInput")
    w2v = nc.dram_tensor("w2v", [64, 16], F16, kind="ExternalInput")
    iota = nc.dram_tensor("iota", [128, P], dt.int16, kind="ExternalInput")
    ends = nc.dram_tensor("ends", [128, bpc], F32, kind="ExternalInput")
    biasenc = nc.dram_tensor("biasenc", [64, 2], F32, kind="ExternalInput")
    b1v = nc.dram_tensor("b1v", [64, 1], F32, kind="ExternalInput")
    logits = nc.dram_tensor("logits", [bpc, 36, 512], F32, kind="ExternalOutput")
    tbl_dram = nc.dram_tensor("tblscratch", [bpc, 128, 128], F16)

    from contextlib import ExitStack

    with tile.TileContext(nc) as tc, ExitStack() as ctx:
        const = ctx.enter_context(tc.tile_pool(name="const", bufs=1))
        WENC = const.tile([L, 8 * 128], F16)
        nc.sync.dma_start(WENC[:], wenc[:])
        WG = const.tile([64, 256], F16)
        nc.sync.dma_start(WG[:], wg[:])
        IDENT = const.tile([64, 64], F16)
        nc.sync.dma_start(IDENT[:], ident[:])
        MDIFF = const.tile([128, 128], F16)
        I128 = const.tile([128, 128], F16)
        WNL = const.tile([128, 64], F16)
        W2V = const.tile([64, 16], F16)
        IOTA = const.tile([128, P], dt.int16)
        ENDS = const.tile([128, bpc], F32)
        BIASENC = const.tile([64, 2], F32)
        nc.sync.dma_start(BIASENC[:], biasenc[:])
        B1V = const.tile([64, 1], F32)
        IDXF = const.tile([128, (P // 16) * bpc], dt.int16)

        # per-batch u-side weight tables (stay in SBUF for all of phase 2)
        wupool = ctx.enter_context(tc.tile_pool(name="wu", bufs=bpc))

        WUs = []
        tpool = ctx.enter_context(tc.tile_pool(name="tp", bufs=3))
        epool = ctx.enter_context(tc.tile_pool(name="ep", bufs=2))
        ps_t = ctx.enter_context(tc.tile_pool(name="pst", bufs=1, space="PSUM"))

        def make_table(b, T=None):
            if T is None:
                T = tpool.tile([L, A * F], F16, tag="T", name="T")
                nc.sync.dma_start(T[:], traj[b])
            TPS = ps_t.tile([128, 384], F32, tag="tps", name="TPS")
            E_ps = TPS[0:64, 0:256]
            Tv = T[:].rearrange("l (a f) -> l f a", f=8)
            for f in range(8):
                nc.tensor.matmul(
                    E_ps[:, 0:128],
                    WENC[:, 128 * f : 128 * f + 64],
                    Tv[:, f, 0:128],
                    start=(f == 0),
                    stop=(f == 7),
                )
            for f in range(8):
                nc.tensor.matmul(
                    E_ps[:, 128:256],
                    WENC[:, 128 * f + 64 : 128 * f + 128],
                    Tv[:, f, 128:256],
                    start=(f == 0),
                    stop=(f == 7),
                )
            EMB = epool.tile([64, A], F16, tag="emb", name="EMB")
            nc.vector.tensor_scalar(
                EMB[:, 0:128], E_ps[:, 0:128], BIASENC[:, 0:1], None, ALU.add
            )
            nc.vector.tensor_scalar(
                EMB[:, 128:256], E_ps[:, 128:256], BIASENC[:, 1:2], None, ALU.add
            )
            SQ = epool.tile([64, A], F16, tag="sq", name="SQ")
            nc.vector.tensor_tensor(SQ[:], EMB[:], EMB[:], ALU.mult)

            TBL_ps = TPS
            nc.tensor.matmul(
                TBL_ps[:, 0:64], EMB[:, 0:128], IDENT[:], start=True, stop=True
            )
            nc.tensor.matmul(
                TBL_ps[:, 64:128], EMB[:, 0:128], WG[:, 0:64],
                start=True, stop=False,
            )
            nc.tensor.matmul(
                TBL_ps[:, 64:128], SQ[:, 0:128], WG[:, 64:128],
                start=False, stop=True,
            )
            nc.tensor.matmul(
                TBL_ps[:, 128:192], EMB[:, 128:256], IDENT[:], start=True, stop=True
            )
            nc.tensor.matmul(
                TBL_ps[:, 192:256], EMB[:, 128:256], WG[:, 128:192],
                start=True, stop=False,
            )
            nc.tensor.matmul(
                TBL_ps[:, 192:256], SQ[:, 128:256], WG[:, 192:256],
                start=False, stop=True,
            )
            TBL = epool.tile([128, 256], F16, tag="tbl", name="TBL")
            nc.scalar.activation(TBL[:], TBL_ps[:, 0:256], AF.Copy)
            # f-half -> DRAM gather table (256B rows)
            nc.sync.dma_start(tbl_dram[b], TBL[:, 0:128])
            # u-half -> differenced weights W~[p] = W[p] - W[p+1]
            nc.tensor.matmul(
                TBL_ps[:, 256:384], MDIFF[:], TBL[:, 128:256], start=True, stop=True
            )
            WU = wupool.tile([128, 128], F16, tag=f"wu{b}", name="WU")
            nc.scalar.activation(WU[:], TBL_ps[:, 256:384], AF.Copy)
            WUs.append(WU)

        # ---------------- phase 2: pair pipeline --------------------------
        gpool = ctx.enter_context(tc.tile_pool(name="gp", bufs=3))
        npool = ctx.enter_context(tc.tile_pool(name="np", bufs=2))
        apool = ctx.enter_context(tc.tile_pool(name="ap", bufs=6))
        dpool = ctx.enter_context(tc.tile_pool(name="dp", bufs=6))
        wpool = ctx.enter_context(tc.tile_pool(name="wp", bufs=4))
        lpool = ctx.enter_context(tc.tile_pool(name="lp", bufs=2))
        ps_p1 = ctx.enter_context(tc.tile_pool(name="psp1", bufs=3, space="PSUM"))
        ps_l = ctx.enter_context(tc.tile_pool(name="psl", bufs=1, space="PSUM"))

        GFs = {}

        def mm512(out_ap, w_ap, x_ap, start, stop, n):
            for i in range(0, n, 512):
                nc.tensor.matmul(
                    out_ap[:, i : i + 512], w_ap, x_ap[:, i : i + 512],
                    start=start, stop=stop,
                )


        def start_gather(b):
            GF = gpool.tile([128, P], F16, tag="gf", name="GF")
            for h in range(GPC):
                nc.gpsimd.dma_gather(
                    GF[:, GW * h : GW * (h + 1)].rearrange("p (c n) -> p c n", c=1),
                    tbl_dram[b],
                    IDXF[:, (P // 16) * b + (GW // 16) * h : (P // 16) * b + (GW // 16) * (h + 1)],
                    num_idxs=GW,
                    num_idxs_reg=GW,
                    elem_size=128,
                    transpose=True,
                    single_packet=False,
                )
            GFs[b] = GF

        LOOK_T = 2
        LOOK_G = 2
        T_pre = []
        for b in range(min(LOOK_T, bpc)):
            T0 = tpool.tile([L, A * F], F16, tag="T", name="T0")
            nc.sync.dma_start(T0[:], traj[b])
            T_pre.append(T0)
        # warm the PE p-state on already-resident weights while traj loads land
        WARM = ps_t.tile([128, 384], F32, tag="tps", name="WARM")
        for w in range(8):
            nc.tensor.matmul(
                WARM[0:64, 0:384], WENC[:, 0:64], WENC[:, 128:512],
                start=True, stop=True,
            )
        nc.sync.dma_start(MDIFF[:], mdiff[:])
        nc.sync.dma_start(I128[:], i128[:])
        nc.sync.dma_start(WNL[:], wnl[:])
        nc.sync.dma_start(W2V[:], w2v[:])
        nc.sync.dma_start(ENDS[:], ends[:])
        nc.sync.dma_start(B1V[:], b1v[:])
        for b in range(min(LOOK_T, bpc)):
            make_table(b, T=T_pre[b])
        nc.sync.dma_start(IDXF[:], idxf[:])
        nc.sync.dma_start(IOTA[:], iota[:])
        for b in range(min(LOOK_G, bpc)):
            start_gather(b)

        for b in range(bpc):
            if b + LOOK_T < bpc:
                make_table(b + LOOK_T)
            if b + LOOK_G < bpc:
                start_gather(b + LOOK_G)
            GF = GFs.pop(b)
            WU = WUs[b]
            NL = npool.tile([128, P], F16, tag="nl", name="NL")
            WH = wpool.tile([64, P], F16, tag="wh", name="WH")
            P_L = ps_l.tile([36, 512], F32, tag="pl", name="P_L")  # rows 0:4 h=0, 32:36 h=1
            for c in range(NCH):
                cs = slice(CH * c, CH * (c + 1))
                A_t = apool.tile([128, CH], F16, tag="a", name="A_t")
                nc.vector.tensor_scalar(
                    A_t[:], IOTA[:, cs], ENDS[:, b : b + 1], 0.0,
                    ALU.subtract, ALU.is_lt,
                )
                P1 = ps_p1.tile([128, CH], F32, tag="p1", name="P1")
                mm512(P1, WU[:], A_t[:], True, True, CH)
                D = dpool.tile([64, CH], F16, tag="d", name="D")
                nc.vector.tensor_tensor(D[:], GF[0:64, cs], P1[0:64, :], ALU.subtract)
                mm512(P1, I128[:], GF[:, cs], False, True, CH)
                nc.scalar.activation(NL[0:64, cs], P1[0:64, :], AF.Square)
                nc.vector.tensor_scalar(
                    NL[64:128, cs].bitcast(dt.int16), D[:].bitcast(dt.int16),
                    32767, None, ALU.bitwise_and,
                )
                mm512(P1[64:128, :], WNL[:], NL[:, cs], False, True, CH)
                nc.scalar.activation(WH[:, cs], P1[64:128, :], AF.Relu, bias=B1V[:])
                for hh in range(2):
                    nc.tensor.matmul(
                        P_L[32 * hh : 32 * hh + 4, :],
                        W2V[:, 4 * c : 4 * (c + 1)],
                        WH[:, CH * c + 512 * hh : CH * c + 512 * (hh + 1)],
                        start=(c == 0), stop=(c == NCH - 1),
                    )
            LE = lpool.tile([36, 512], F32, tag="le", name="LE")
            nc.vector.tensor_scalar(LE[:], P_L[:], 0.0, None, ALU.add)
            nc.sync.dma_start(logits[b], LE[:])

    nc.compile()
    return nc


def prep_inputs(inputs, bpc=BPC, ncores=NCORES):
    f16 = np.float16
    traj = np.asarray(inputs["batch_trajectories"], np.float32)
    pairs = np.asarray(inputs["pairs"], np.int32)
    enc_f_W = np.asarray(inputs["enc_f_W"], np.float32)
    enc_u_W = np.asarray(inputs["enc_u_W"], np.float32)
    cls_W1 = np.asarray(inputs["cls_W1"], np.float32)
    cls_W2 = np.asarray(inputs["cls_W2"], np.float32)

    wenc = np.zeros((L, 8, 128), np.float32)
    wenc[:, :, 0:64] = enc_f_W.reshape(L, 8, E)
    wenc[:, :, 64:128] = enc_u_W.reshape(L, 8, E)
    wenc = wenc.reshape(L, 8 * 128).astype(f16)

    W1a, W1b, W1c, W1d = (cls_W1[i * E : (i + 1) * E] for i in range(4))
    wd2 = 0.5 * W1d
    wg = np.concatenate([W1a, -wd2, W1b, -wd2], axis=1).astype(f16)  # [64, 256]

    wnl = np.concatenate([wd2, W1c], axis=0).astype(f16)  # [128, 64]
    w2v = np.zeros((64, 16), np.float32)
    for c in range(4):
        w2v[:, 4 * c + c] = cls_W2[:, 0]
    w2v = w2v.astype(f16)  # [64, 16]

    ident = np.eye(64, dtype=f16)
    i128 = np.eye(128, dtype=f16)
    mdiff = (np.eye(128) - np.eye(128, k=-1)).astype(f16)  # col m: +1@m, -1@m+1

    iota = np.tile(np.arange(P, dtype=np.int16), (128, 1))

    biasenc = np.stack(
        [np.asarray(inputs["enc_f_b"], np.float32), np.asarray(inputs["enc_u_b"], np.float32)],
        axis=1,
    )
    b1v = np.asarray(inputs["cls_b1"], np.float32).reshape(64, 1)

    shared = {
        "wenc": wenc, "wg": wg, "ident": ident, "mdiff": mdiff, "i128": i128,
        "wnl": wnl, "w2v": w2v, "iota": iota, "biasenc": biasenc, "b1v": b1v,
    }

    in_maps = []
    perms = []
    for cix in range(ncores):
        bs = slice(cix * bpc, (cix + 1) * bpc)
        idx_cols = []
        ends_cols = []
        core_perms = []
        for b in range(cix * bpc, (cix + 1) * bpc):
            f_idx = pairs[b, :, 0]
            u_idx = pairs[b, :, 1] - NF
            perm = np.argsort(u_idx, kind="stable")
            core_perms.append(perm)
            f_sorted = f_idx[perm].astype(np.int16)
            cnt = np.bincount(u_idx, minlength=128).astype(np.int64)
            ends = np.cumsum(cnt).astype(np.float32)  # end[p] = start[p]+cnt[p]
            ends_cols.append(ends)
            w16 = np.tile(f_sorted.reshape(P // 16, 16).T, (8, 1))  # [128, P//16]
            idx_cols.append(w16)
        m = dict(shared)
        m["traj"] = np.ascontiguousarray(
            traj[bs].reshape(bpc, L, A * F).astype(np.float16)
        )
        m["idxf"] = np.concatenate(idx_cols, axis=1)
        m["ends"] = np.stack(ends_cols, axis=1)  # [128, bpc]
        in_maps.append(m)
        perms.append(core_perms)
    return in_maps, perms


_PROGRAM_CACHE = {}


def kernel(**inputs):
    bpc, ncores = BPC, NCORES
    key = (bpc, ncores)
    if key not in _PROGRAM_CACHE:
        _PROGRAM_CACHE[key] = build_program(bpc)
    nc = _PROGRAM_CACHE[key]
    in_maps, perms = prep_inputs(inputs, bpc, ncores)
    res = run_bass_kernel_spmd(nc, in_maps, core_ids=list(range(ncores)))
    b2 = float(np.asarray(inputs["cls_b2"], np.float32).reshape(-1)[0])
    out = np.empty((B, P, 1), np.float32)
    for cix in range(ncores):
        raw = res.results[cix]["logits"]  # [bpc, 36, 512]
        lg = np.empty((bpc, P), np.float32)
        for c4 in range(4):
            for hh in range(2):
                lg[:, 1024 * c4 + 512 * hh : 1024 * c4 + 512 * (hh + 1)] = raw[:, 32 * hh + c4, :]
        for b in range(bpc):
            perm = perms[cix][b]
            row = np.empty(P, np.float32)
            row[perm] = lg[b]
            out[cix * bpc + b, :, 0] = row + b2
    return out


if __name__ == "__main__":
    rng = np.random.default_rng(0)
    ins = {
        "batch_trajectories": rng.standard_normal((B, L, A, F)).astype(np.float32),
        "batch_roles": np.zeros((B, A), np.int32),
        "pairs": np.stack(
            [rng.integers(0, NF, (B, P)), rng.integers(NF, A, (B, P))], axis=-1
        ).astype(np.int32),
        "enc_f_W": (rng.standard_normal((L * F, E)) / 20).astype(np.float32),
        "enc_f_b": np.zeros(E, np.float32),
        "enc_u_W": (rng.standard_normal((L * F, E)) / 20).astype(np.float32),
        "enc_u_b": np.zeros(E, np.float32),
        "cls_W1": (rng.standard_normal((4 * E, E)) / 16).astype(np.float32),
        "cls_b1": np.zeros(E, np.float32),
        "cls_W2": (rng.standard_normal((E, 1)) / 8).astype(np.float32),
        "cls_b2": np.zeros(1, np.float32),
    }
    out = kernel(**ins)
    # numpy reference
    x = ins["batch_trajectories"].transpose(0, 2, 1, 3).reshape(B, A, L * F)
    ef_all = x[:, :NF] @ ins["enc_f_W"] + ins["enc_f_b"]
    eu_all = x[:, NF:] @ ins["enc_u_W"] + ins["enc_u_b"]
    fi = ins["pairs"][..., 0]
    ui = ins["pairs"][..., 1] - NF
    ef = np.take_along_axis(ef_all, fi[..., None], 1)
    eu = np.take_along_axis(eu_all, ui[..., None], 1)
    rel = np.concatenate([ef, eu, np.abs(ef - eu), ef * eu], -1)
    h = np.maximum(rel @ ins["cls_W1"] + ins["cls_b1"], 0)
    ref = h @ ins["cls_W2"] + ins["cls_b2"]
    err = np.linalg.norm(out - ref) / np.linalg.norm(ref)
    print("rel err:", err)

